# revision 1
# baseline (speedup 1.0000x reference)
"""Trainium2 Bass kernel for nn_DecompMultiTransform (RGCN basis-decomposition).

Reference computation:
    full_w = (w_comp @ weight).reshape(64, 256, 256)   # per-type weights
    out[n, :] = x[n, :] @ full_w[xtype[n]]             # N = 4096

Kernel formulation (avoids materializing the 16 MB full_w and the 1 GB
per-sample weight gather):
    onehot[t, n] = (xtype[n] == t)                     # [64, 512] per core
    cb_b[p, n]   = w_comp[:, b]^T @ onehot             # = w_comp[xtype[n], b]
    u_b[p, ihn]  = x^T * cb_b                          # scaled x halves
    outT[o, n]   = sum_{b,i} weight[b, i*256+o] * u
i.e. one dense K=4096 float32r matmul per core after a cheap on-device
type-lookup (broadcast-compare-matmul) and elementwise scale.

Sharding: data-parallel over N across 8 cores (512 rows each); weight and
w_comp replicated (w_comp uploaded column-replicated so each basis column
can be used as a stationary matmul operand). x is uploaded transposed and
the output comes back transposed - pure layout choices done at shard time
on host. All math (type lookup, scaling, matmuls) runs on device.

Matmuls run in float32r (TRN2's full-rate fp32 mode, ~1.2e-4 rounding).
"""

import sys

if "/opt/trn_rl_repo" not in sys.path:
    sys.path.insert(0, "/opt/trn_rl_repo")

import numpy as np

import concourse.bass as bass
import concourse.mybir as mybir
import concourse.tile as tile
from concourse import bacc
from concourse.bass_utils import run_bass_kernel_spmd

P = 128
N_FULL = 4096
IN_DIM = 256
OUT_DIM = 256
NUM_B = 16
NUM_T = 64
N_CORES = 8
ROWS = N_FULL // N_CORES          # 512 rows per core
KT = NUM_B * (IN_DIM // P)        # 32 contraction tiles of 128
GPS_BASES = frozenset({3, 7, 11, 14})  # bases whose scale-TT runs on gpsimd

F32 = mybir.dt.float32
F32R = mybir.dt.float32r
I32 = mybir.dt.int32


def _build_program():
    nc = bacc.Bacc("TRN2", target_bir_lowering=False, debug=False)

    xT = nc.declare_dram_parameter("xT", [P, 2 * ROWS], F32, isOutput=False)
    xtype = nc.declare_dram_parameter("xtype", [ROWS], I32, isOutput=False)
    iota_in = nc.declare_dram_parameter("iota_in", [NUM_T, 1], I32, isOutput=False)
    wcomp_bc = nc.declare_dram_parameter("wcomp_bc", [NUM_T, NUM_B * P], F32R, isOutput=False)
    weight = nc.declare_dram_parameter("weight", [NUM_B, IN_DIM * OUT_DIM], F32R, isOutput=False)
    outT = nc.declare_dram_parameter("outT", [OUT_DIM, ROWS], F32, isOutput=True)

    # weight chunk per b: [128, 2, 256], w_chunk[b][p, ih, o] = weight[b, (ih*128+p)*256 + o]
    wv = weight.ap().rearrange("b (ih p o) -> b p ih o", ih=2, p=P, o=OUT_DIM)

    with tile.TileContext(nc) as tc:
        with (
            tc.tile_pool(name="const", bufs=1) as constp,
            tc.tile_pool(name="wpool", bufs=1) as wpool,
            tc.tile_pool(name="cbp", bufs=2) as cbp,
            tc.tile_pool(name="up", bufs=5) as up,
            tc.tile_pool(name="outp", bufs=2) as outp,
            tc.tile_pool(name="psb", bufs=5, space="PSUM") as psb,
            tc.tile_pool(name="pso", bufs=1, space="PSUM") as pso,
        ):
            # ---- tiny inputs first: type ids (partition-broadcast), iota ----
            xtypeB = constp.tile([NUM_T, ROWS], I32, name="xtypeB")
            xtype_bcast = bass.AP(
                tensor=xtype.ap().tensor,
                offset=0,
                ap=[[0, NUM_T], [1, ROWS]],
            )
            nc.sync.dma_start(out=xtypeB[:], in_=xtype_bcast)
            iota_c = constp.tile([NUM_T, 1], I32, name="iota_c")
            nc.sync.dma_start(out=iota_c[:], in_=iota_in.ap()[:, :])

            wcb = constp.tile([NUM_T, NUM_B * P], F32R, name="wcb")
            nc.sync.dma_start(out=wcb[:], in_=wcomp_bc.ap()[:, :])

            xtcat = constp.tile([P, 2 * ROWS], F32, name="xtcat")
            nc.scalar.dma_start(out=xtcat, in_=xT.ap()[:, :])

            # weight chunks, resident; split across the two HWDGE queues
            wts = []
            for b in range(NUM_B):
                wt = wpool.tile([P, 2, OUT_DIM], F32R, name=f"w{b}")
                eng = nc.sync if b % 2 == 0 else nc.scalar
                eng.dma_start(out=wt, in_=wv[b])
                wts.append(wt)

            # ---- onehot[t, n] = (iota[t] == xtype[n]) ----
            onehot = constp.tile([NUM_T, ROWS], F32R, name="onehot")
            nc.vector.tensor_tensor(
                out=onehot[:],
                in0=iota_c[:].to_broadcast([NUM_T, ROWS]),
                in1=xtypeB[:],
                op=mybir.AluOpType.is_equal,
            )

            # ---- per-basis: cb = w_comp[:,b]-bcast ^T @ onehot; scale; matmul ----
            psums = [
                pso.tile([P, ROWS], F32, name=f"out{oh}", space="PSUM")
                for oh in range(2)
            ]

            def emit_cb(b):
                cb_ps = psb.tile([P, ROWS], F32, name="cbps", tag="cbps", space="PSUM")
                nc.tensor.matmul(
                    out=cb_ps[:],
                    lhsT=wcb[:, b * P : (b + 1) * P],
                    rhs=onehot[:],
                    start=True,
                    stop=True,
                )
                if b in GPS_BASES:
                    # gpsimd cannot read PSUM; stage via the scalar engine
                    cb_sb = cbp.tile([P, ROWS], F32, name="cbsb", tag="cbsb")
                    nc.scalar.copy(cb_sb[:], cb_ps[:])
                    return cb_sb
                return cb_ps

            cbs = {0: emit_cb(0), 1: emit_cb(1)}
            for b in range(NUM_B):
                if b + 2 < NUM_B:
                    cbs[b + 2] = emit_cb(b + 2)
                cb_src = cbs.pop(b)
                cb_rep = cb_src[:].rearrange("p (one n) -> p one n", one=1).to_broadcast(
                    [P, 2, ROWS]
                )
                u = up.tile([P, 2 * ROWS], F32R, name="u", tag="u")
                eng = nc.gpsimd if b in GPS_BASES else nc.vector
                eng.tensor_tensor(
                    out=u[:].rearrange("p (ih n) -> p ih n", ih=2),
                    in0=xtcat[:].rearrange("p (ih n) -> p ih n", ih=2),
                    in1=cb_rep,
                    op=mybir.AluOpType.mult,
                )
                for ih in range(2):
                    kt = b * 2 + ih
                    for oh in range(2):
                        nc.tensor.matmul(
                            out=psums[oh][:],
                            lhsT=wts[b][:, ih, oh * P : (oh + 1) * P],
                            rhs=u[:, ih * ROWS : (ih + 1) * ROWS],
                            start=(kt == 0),
                            stop=(kt == KT - 1),
                        )

            # ---- drain outT ----
            for oh in range(2):
                ot = outp.tile([P, ROWS], F32, name=f"ot{oh}")
                nc.scalar.copy(ot[:], psums[oh][:])
                eng = nc.sync if oh == 0 else nc.scalar
                eng.dma_start(out=outT.ap()[oh * P : (oh + 1) * P, :], in_=ot)

    nc.compile()
    return nc


_PROGRAM = None
LAST_RESULT = None  # test harness introspection


def kernel(x, xtype, weight, w_comp, trace=False):
    global _PROGRAM, LAST_RESULT
    x = np.asarray(x, dtype=np.float32)
    xtype = np.asarray(xtype)
    weight = np.asarray(weight, dtype=np.float32)
    w_comp = np.asarray(w_comp, dtype=np.float32)
    assert x.shape == (N_FULL, IN_DIM) and weight.shape == (NUM_B, IN_DIM * OUT_DIM)

    if _PROGRAM is None:
        _PROGRAM = _build_program()
    nc = _PROGRAM

    xtype32 = xtype.astype(np.int32)
    iota_c = np.arange(NUM_T, dtype=np.int32).reshape(NUM_T, 1)
    # w_comp columns replicated so each [64, 128] slice is a constant column
    wcomp_bc = np.ascontiguousarray(np.repeat(w_comp, P, axis=1))  # [64, 16*128]
    in_maps = []
    for c in range(N_CORES):
        s = slice(c * ROWS, (c + 1) * ROWS)
        in_maps.append(
            {
                "xT": np.ascontiguousarray(
                    x[s].T.reshape(2, P, ROWS).transpose(1, 0, 2).reshape(P, 2 * ROWS)
                ),
                "xtype": np.ascontiguousarray(xtype32[s]),
                "iota_in": iota_c,
                "wcomp_bc": wcomp_bc,
                "weight": weight,
            }
        )

    res = run_bass_kernel_spmd(nc, in_maps, list(range(N_CORES)), trace=trace)
    LAST_RESULT = res

    out = np.empty((N_FULL, OUT_DIM), np.float32)
    for c in range(N_CORES):
        s = slice(c * ROWS, (c + 1) * ROWS)
        out[s] = res.results[c]["outT"].T
    return out



# revision 4
# speedup vs baseline: 1.0256x; 1.0256x over previous
"""Trainium2 Bass kernel for nn_DecompMultiTransform (RGCN basis-decomposition).

Reference computation:
    full_w = (w_comp @ weight).reshape(64, 256, 256)   # per-type weights
    out[n, :] = x[n, :] @ full_w[xtype[n]]             # N = 4096

Kernel formulation (avoids materializing the 16 MB full_w):
    onehot[t, n] = (xtype[n] == t)                     # [64, 512] per core
    cb_b[p, n]   = w_comp[:, b]^T @ onehot             # = w_comp[xtype[n], b]
    u_b[p, ihn]  = x^T * cb_b                          # scaled x halves
    outT[o, n]   = sum_{b,ih} weight-chunk^T @ u_b     # K=128 dense matmuls

Sharding: data-parallel over N across 8 cores (512 rows each); weight and
w_comp replicated. All math (type lookup, scaling, matmuls) runs on device;
host only does layout (transpose/pack) and dtype casts.

v2 changes vs v1 (53.3us):
  - bf16 matmul operands (halves weight DMA + LDWEIGHTS, full PE rate),
    fp32 PSUM accumulation. Measured numerics: rel err ~3.6e-3.
  - weight upload: 4 batched DMAs in PE-ready packed layout (was 16).
  - all scale ops on DVE in 16-bit fast mode; cb staged PSUM->SBUF bf16 on
    the scalar engine (gpsimd tensor_tensor was 2.4us/op - dropped).
  - cb matmuls emitted 3 bases ahead so the PE never starves (p-state ramp).
"""

import sys

if "/opt/trn_rl_repo" not in sys.path:
    sys.path.insert(0, "/opt/trn_rl_repo")

import numpy as np
import ml_dtypes

import concourse.bass as bass
import concourse.mybir as mybir
import concourse.tile as tile
from concourse import bacc
from concourse.bass_utils import run_bass_kernel_spmd

P = 128
N_FULL = 4096
IN_DIM = 256
OUT_DIM = 256
NUM_B = 16
NUM_T = 64
N_CORES = 8
ROWS = N_FULL // N_CORES          # 512 rows per core
KT = NUM_B * (IN_DIM // P)        # 32 contraction tiles of 128
W_BATCHES = (2, 2, 4, 8)          # weight DMA batch sizes (bases)
CB_AHEAD = 3                      # cb pipeline lookahead (bases)

F32 = mybir.dt.float32
BF16 = mybir.dt.bfloat16
I32 = mybir.dt.int32
NPBF16 = ml_dtypes.bfloat16


def _build_program():
    nc = bacc.Bacc("TRN2", target_bir_lowering=False, debug=False)

    xT = nc.declare_dram_parameter("xT", [P, 2 * ROWS], BF16, isOutput=False)
    xtype = nc.declare_dram_parameter("xtype", [ROWS], I32, isOutput=False)
    wcb = nc.declare_dram_parameter("wcb", [NUM_T, NUM_B * P], BF16, isOutput=False)
    # packed weight: wt[p, b, ih, o] = weight[b, (ih*128+p)*256 + o]
    wt = nc.declare_dram_parameter("wt", [P, NUM_B * 2 * OUT_DIM], BF16, isOutput=False)
    outT = nc.declare_dram_parameter("outT", [OUT_DIM, ROWS], F32, isOutput=True)

    wtv = wt.ap().rearrange("p (b r) -> p b r", b=NUM_B)

    with tile.TileContext(nc) as tc:
        with (
            tc.tile_pool(name="const", bufs=1) as constp,
            tc.tile_pool(name="wpool", bufs=1) as wpool,
            tc.tile_pool(name="cbp", bufs=4) as cbp,
            tc.tile_pool(name="up", bufs=4) as up,
            tc.tile_pool(name="outp", bufs=1) as outp,
            tc.tile_pool(name="psb", bufs=4, space="PSUM") as psb,
            tc.tile_pool(name="pso", bufs=1, space="PSUM") as pso,
        ):
            # ---- type ids, partition-broadcast [64, 512]; iota on DVE ----
            xtypeB = constp.tile([NUM_T, ROWS], I32, name="xtypeB")
            xtype_bcast = bass.AP(
                tensor=xtype.ap().tensor,
                offset=0,
                ap=[[0, NUM_T], [1, ROWS]],
            )
            nc.sync.dma_start(out=xtypeB[:], in_=xtype_bcast)

            iota_c = constp.tile([NUM_T, 1], I32, name="iota_c")
            nc.gpsimd.iota(iota_c[:], [[0, 1]], channel_multiplier=1)

            wcb_t = constp.tile([NUM_T, NUM_B * P], BF16, name="wcb_t")
            nc.sync.dma_start(out=wcb_t[:], in_=wcb.ap()[:, :])

            xtcat = constp.tile([P, 2 * ROWS], BF16, name="xtcat")
            nc.sync.dma_start(out=xtcat, in_=xT.ap()[:, :])

            # weight batches in consumption order: first batch on the sync
            # queue (behind the critical-path smalls), bulk on scalar
            wtbs = []
            b0 = 0
            for k, nb in enumerate(W_BATCHES):
                wtb = wpool.tile([P, nb, 2, 2, P], BF16, name=f"wtb{k}")
                eng = nc.sync if k == 0 else nc.scalar
                eng.dma_start(
                    out=wtb,
                    in_=wtv[:, b0 : b0 + nb, :].rearrange(
                        "p b (ih oh q) -> p b ih oh q", ih=2, oh=2, q=P
                    ),
                )
                wtbs.append((b0, wtb))
                b0 += nb

            def wslice(b, ih, oh):
                for b0, wtb in reversed(wtbs):
                    if b >= b0:
                        return wtb[:, b - b0, ih, oh, :]
                raise AssertionError

            # ---- onehot[t, n] = (iota[t] == xtype[n]), bf16 ----
            onehot = constp.tile([NUM_T, ROWS], BF16, name="onehot")
            nc.vector.tensor_tensor(
                out=onehot[:],
                in0=iota_c[:].to_broadcast([NUM_T, ROWS]),
                in1=xtypeB[:],
                op=mybir.AluOpType.is_equal,
            )

            # ---- per-basis pipeline ----
            psums = [
                pso.tile([P, ROWS], F32, name=f"out{oh}", space="PSUM")
                for oh in range(2)
            ]

            def emit_cb(b):
                cb_ps = psb.tile([P, ROWS], F32, name="cbps", tag="cbps", space="PSUM")
                nc.tensor.matmul(
                    out=cb_ps[:],
                    lhsT=wcb_t[:, b * P : (b + 1) * P],
                    rhs=onehot[:],
                    start=True,
                    stop=True,
                )
                cb_sb = cbp.tile([P, ROWS], BF16, name="cbsb", tag="cbsb")
                nc.scalar.copy(cb_sb[:], cb_ps[:])
                return cb_sb

            cbs = {b: emit_cb(b) for b in range(CB_AHEAD)}
            for b in range(NUM_B):
                if b + CB_AHEAD < NUM_B:
                    cbs[b + CB_AHEAD] = emit_cb(b + CB_AHEAD)
                cb_sb = cbs.pop(b)
                cb_rep = cb_sb[:].rearrange("p (one n) -> p one n", one=1).to_broadcast(
                    [P, 2, ROWS]
                )
                u = up.tile([P, 2 * ROWS], BF16, name="u", tag="u")
                nc.vector.tensor_tensor(
                    out=u[:].rearrange("p (ih n) -> p ih n", ih=2),
                    in0=xtcat[:].rearrange("p (ih n) -> p ih n", ih=2),
                    in1=cb_rep,
                    op=mybir.AluOpType.mult,
                )
                for ih in range(2):
                    kt = b * 2 + ih
                    for oh in range(2):
                        nc.tensor.matmul(
                            out=psums[oh][:],
                            lhsT=wslice(b, ih, oh),
                            rhs=u[:, ih * ROWS : (ih + 1) * ROWS],
                            start=(kt == 0),
                            stop=(kt == KT - 1),
                        )

            # ---- drain outT ----
            for oh in range(2):
                ot = outp.tile([P, ROWS], F32, name=f"ot{oh}")
                nc.scalar.copy(ot[:], psums[oh][:])
                eng = nc.sync if oh == 0 else nc.scalar
                eng.dma_start(out=outT.ap()[oh * P : (oh + 1) * P, :], in_=ot)

    nc.compile()
    return nc


_PROGRAM = None
LAST_RESULT = None  # test harness introspection


def kernel(x, xtype, weight, w_comp, trace=False):
    global _PROGRAM, LAST_RESULT
    x = np.asarray(x, dtype=np.float32)
    xtype = np.asarray(xtype)
    weight = np.asarray(weight, dtype=np.float32)
    w_comp = np.asarray(w_comp, dtype=np.float32)
    assert x.shape == (N_FULL, IN_DIM) and weight.shape == (NUM_B, IN_DIM * OUT_DIM)

    if _PROGRAM is None:
        _PROGRAM = _build_program()
    nc = _PROGRAM

    xtype32 = xtype.astype(np.int32)
    # w_comp columns replicated so each [64, 128] slice is a constant column
    wcb_host = np.ascontiguousarray(np.repeat(w_comp, P, axis=1)).astype(NPBF16)
    # packed weight [p, b, ih, o]
    wt_host = np.ascontiguousarray(
        weight.reshape(NUM_B, 2, P, OUT_DIM).transpose(2, 0, 1, 3).reshape(P, -1)
    ).astype(NPBF16)
    in_maps = []
    for c in range(N_CORES):
        s = slice(c * ROWS, (c + 1) * ROWS)
        in_maps.append(
            {
                "xT": np.ascontiguousarray(
                    x[s].T.reshape(2, P, ROWS).transpose(1, 0, 2).reshape(P, 2 * ROWS)
                ).astype(NPBF16),
                "xtype": np.ascontiguousarray(xtype32[s]),
                "wcb": wcb_host,
                "wt": wt_host,
            }
        )

    res = run_bass_kernel_spmd(nc, in_maps, list(range(N_CORES)), trace=trace)
    LAST_RESULT = res

    out = np.empty((N_FULL, OUT_DIM), np.float32)
    for c in range(N_CORES):
        s = slice(c * ROWS, (c + 1) * ROWS)
        out[s] = res.results[c]["outT"].T
    return out


# revision 5
# speedup vs baseline: 1.1799x; 1.1505x over previous
"""Trainium2 Bass kernel for nn_DecompMultiTransform (RGCN basis-decomposition).

Reference computation:
    full_w = (w_comp @ weight).reshape(64, 256, 256)   # per-type weights
    out[n, :] = x[n, :] @ full_w[xtype[n]]             # N = 4096

Kernel formulation (avoids materializing the 16 MB full_w):
    onehot[t, n] = (xtype[n] == t)                     # [64, 512] per core
    cb_b[n]      = w_comp[xtype[n], b]  (via matmul with onehot)
    u_b[p, ihn]  = x^T * cb_b                          # scaled x halves
    outT[o, n]   = sum_{b,ih} weight-chunk^T @ u_b     # K=128 dense matmuls

Sharding: data-parallel over N across 8 cores (512 rows each); weight and
w_comp replicated. All math (type lookup, scaling, matmuls) runs on device;
host only does layout (transpose/pack) and dtype casts.

v3 changes vs v2 (51.9us):
  - PE warmup: dummy matmuls on memset tiles run during the framework
    preamble/input-DMA window so the tensor engine is at full p-state when
    real matmuls start (measured: first matmuls ran 760ns vs 454ns late).
  - cb mostly off the PE: one [16,512] matmul computes all 16 cb rows; they
    are partition-broadcast via a 16KB DRAM bounce + stride-0 broadcast
    reads (3 grouped DMAs). Only bases 0-1 keep the v2 PE-broadcast path to
    hide the bounce latency. PE: 80 -> 68 matmuls.
  - w_comp uploaded as 2KB [64,16] (was a 256KB column-replicated copy);
    the 2-base replication for the PE path is built on-device by the DVE.
  - output drain split across scalar + vector engines.
"""

import sys

if "/opt/trn_rl_repo" not in sys.path:
    sys.path.insert(0, "/opt/trn_rl_repo")

import numpy as np
import ml_dtypes

import concourse.bass as bass
import concourse.mybir as mybir
import concourse.tile as tile
from concourse import bacc
from concourse.bass_utils import run_bass_kernel_spmd

P = 128
N_FULL = 4096
IN_DIM = 256
OUT_DIM = 256
NUM_B = 16
NUM_T = 64
N_CORES = 8
ROWS = N_FULL // N_CORES          # 512 rows per core
KT = NUM_B * (IN_DIM // P)        # 32 contraction tiles of 128
W_BATCHES = (2, 2, 4, 8)          # weight DMA batch sizes (bases)
PE_BASES = 2                      # head bases with PE-broadcast cb
CB_GROUPS = ((2, 6), (6, 11), (11, 16))  # DRAM-bounce broadcast groups
N_DUMMY = 5                       # PE p-state warmup matmuls

F32 = mybir.dt.float32
BF16 = mybir.dt.bfloat16
I32 = mybir.dt.int32
NPBF16 = ml_dtypes.bfloat16


def _build_program():
    nc = bacc.Bacc("TRN2", target_bir_lowering=False, debug=False)

    xT = nc.declare_dram_parameter("xT", [P, 2 * ROWS], BF16, isOutput=False)
    xtype = nc.declare_dram_parameter("xtype", [ROWS], I32, isOutput=False)
    wcomp = nc.declare_dram_parameter("wcomp", [NUM_T, NUM_B], BF16, isOutput=False)
    # packed weight: wt[p, b, ih, o] = weight[b, (ih*128+p)*256 + o]
    wt = nc.declare_dram_parameter("wt", [P, NUM_B * 2 * OUT_DIM], BF16, isOutput=False)
    outT = nc.declare_dram_parameter("outT", [OUT_DIM, ROWS], F32, isOutput=True)

    cbdram = nc.dram_tensor("cb_bounce", [NUM_B, ROWS], BF16)
    wtv = wt.ap().rearrange("p (b r) -> p b r", b=NUM_B)

    with tile.TileContext(nc) as tc:
        with (
            tc.tile_pool(name="const", bufs=1) as constp,
            tc.tile_pool(name="wpool", bufs=1) as wpool,
            tc.tile_pool(name="cbp", bufs=2) as cbp,
            tc.tile_pool(name="up", bufs=4) as up,
            tc.tile_pool(name="outp", bufs=1) as outp,
            tc.tile_pool(name="psb", bufs=2, space="PSUM") as psb,
            tc.tile_pool(name="pso", bufs=1, space="PSUM") as pso,
        ):
            # ---- PE warmup: memset scratch, then dummy matmuls ----
            dlhs = constp.tile([P, P], BF16, name="dlhs")
            drhs = constp.tile([P, ROWS], BF16, name="drhs")
            nc.vector.memset(dlhs[:], 0)
            nc.vector.memset(drhs[:], 0)
            dps = pso.tile([P, ROWS], F32, name="dps", space="PSUM")
            for _ in range(N_DUMMY):
                nc.tensor.matmul(
                    out=dps[:], lhsT=dlhs[:], rhs=drhs[:], start=True, stop=True
                )

            # ---- sync-queue DMAs in priority order ----
            xtypeB = constp.tile([NUM_T, ROWS], I32, name="xtypeB")
            xtype_bcast = bass.AP(
                tensor=xtype.ap().tensor,
                offset=0,
                ap=[[0, NUM_T], [1, ROWS]],
            )
            nc.sync.dma_start(out=xtypeB[:], in_=xtype_bcast)

            wcomp_sb = constp.tile([NUM_T, NUM_B], BF16, name="wcomp_sb")
            nc.sync.dma_start(out=wcomp_sb[:], in_=wcomp.ap()[:, :])

            wtbs = []
            b0 = 0
            for k, nb in enumerate(W_BATCHES):
                wtb = wpool.tile([P, nb, 2, 2, P], BF16, name=f"wtb{k}")
                eng = nc.sync if k == 0 else nc.scalar
                eng.dma_start(
                    out=wtb,
                    in_=wtv[:, b0 : b0 + nb, :].rearrange(
                        "p b (ih oh q) -> p b ih oh q", ih=2, oh=2, q=P
                    ),
                )
                wtbs.append((b0, wtb))
                b0 += nb

            def wslice(b, ih, oh):
                for bb0, wtb in reversed(wtbs):
                    if b >= bb0:
                        return wtb[:, b - bb0, ih, oh, :]
                raise AssertionError

            xtcat = constp.tile([P, 2 * ROWS], BF16, name="xtcat")
            nc.sync.dma_start(out=xtcat, in_=xT.ap()[:, :])

            # ---- iota + onehot ----
            iota_c = constp.tile([NUM_T, 1], I32, name="iota_c")
            nc.gpsimd.iota(iota_c[:], [[0, 1]], channel_multiplier=1)

            onehot = constp.tile([NUM_T, ROWS], BF16, name="onehot")
            nc.vector.tensor_tensor(
                out=onehot[:],
                in0=iota_c[:].to_broadcast([NUM_T, ROWS]),
                in1=xtypeB[:],
                op=mybir.AluOpType.is_equal,
            )

            # ---- on-device column replication for the PE-broadcast bases ----
            wcb2 = constp.tile([NUM_T, PE_BASES, P], BF16, name="wcb2")
            nc.vector.tensor_copy(
                out=wcb2[:],
                in_=wcomp_sb[:, 0:PE_BASES]
                .rearrange("t (b one) -> t b one", one=1)
                .to_broadcast([NUM_T, PE_BASES, P]),
            )

            # ---- cb_all: one matmul computes w_comp[xtype[n], b] for all b ----
            cb_all_ps = pso.tile([NUM_B, ROWS], F32, name="cb_all_ps", space="PSUM")
            nc.tensor.matmul(
                out=cb_all_ps[:],
                lhsT=wcomp_sb[:],
                rhs=onehot[:],
                start=True,
                stop=True,
            )
            cb_all_sb = constp.tile([NUM_B, ROWS], BF16, name="cb_all_sb")
            nc.scalar.copy(cb_all_sb[:], cb_all_ps[:])
            # bounce to DRAM, then partition-broadcast reads (FIFO on sync q)
            nc.sync.dma_start(out=cbdram.ap()[:, :], in_=cb_all_sb[:])
            cbcats = []
            for g, (gb0, gb1) in enumerate(CB_GROUPS):
                nb = gb1 - gb0
                cbc = constp.tile([P, nb, ROWS], BF16, name=f"cbcat{g}")
                src = bass.AP(
                    tensor=cbdram.ap().tensor,
                    offset=gb0 * ROWS,
                    ap=[[0, P], [ROWS, nb], [1, ROWS]],
                )
                nc.sync.dma_start(out=cbc, in_=src)
                cbcats.append((gb0, cbc))

            def cb_rep(b):
                for gb0, cbc in reversed(cbcats):
                    if b >= gb0:
                        return cbc[:, b - gb0 : b - gb0 + 1, :].to_broadcast(
                            [P, 2, ROWS]
                        )
                raise AssertionError

            # ---- head bases via PE broadcast (hides bounce latency) ----
            head_cb = []
            for b in range(PE_BASES):
                cb_ps = psb.tile([P, ROWS], F32, name="cbps", tag="cbps", space="PSUM")
                nc.tensor.matmul(
                    out=cb_ps[:],
                    lhsT=wcb2[:, b, :],
                    rhs=onehot[:],
                    start=True,
                    stop=True,
                )
                cb_sb = cbp.tile([P, ROWS], BF16, name="cbsb", tag="cbsb")
                nc.scalar.copy(cb_sb[:], cb_ps[:])
                head_cb.append(cb_sb)

            # ---- per-basis scale + main matmuls ----
            psums = [
                pso.tile([P, ROWS], F32, name=f"out{oh}", space="PSUM")
                for oh in range(2)
            ]
            for b in range(NUM_B):
                if b < PE_BASES:
                    rep = head_cb[b][:].rearrange(
                        "p (one n) -> p one n", one=1
                    ).to_broadcast([P, 2, ROWS])
                else:
                    rep = cb_rep(b)
                u = up.tile([P, 2 * ROWS], BF16, name="u", tag="u")
                nc.vector.tensor_tensor(
                    out=u[:].rearrange("p (ih n) -> p ih n", ih=2),
                    in0=xtcat[:].rearrange("p (ih n) -> p ih n", ih=2),
                    in1=rep,
                    op=mybir.AluOpType.mult,
                )
                for ih in range(2):
                    kt = b * 2 + ih
                    for oh in range(2):
                        nc.tensor.matmul(
                            out=psums[oh][:],
                            lhsT=wslice(b, ih, oh),
                            rhs=u[:, ih * ROWS : (ih + 1) * ROWS],
                            start=(kt == 0),
                            stop=(kt == KT - 1),
                        )

            # ---- drain outT: oh0 via scalar, oh1 via vector (parallel) ----
            ot0 = outp.tile([P, ROWS], F32, name="ot0")
            nc.scalar.copy(ot0[:], psums[0][:])
            nc.sync.dma_start(out=outT.ap()[0:P, :], in_=ot0)
            ot1 = outp.tile([P, ROWS], F32, name="ot1")
            nc.vector.tensor_copy(out=ot1[:], in_=psums[1][:])
            nc.scalar.dma_start(out=outT.ap()[P : 2 * P, :], in_=ot1)

    nc.compile()
    return nc


_PROGRAM = None
LAST_RESULT = None  # test harness introspection


def kernel(x, xtype, weight, w_comp, trace=False):
    global _PROGRAM, LAST_RESULT
    x = np.asarray(x, dtype=np.float32)
    xtype = np.asarray(xtype)
    weight = np.asarray(weight, dtype=np.float32)
    w_comp = np.asarray(w_comp, dtype=np.float32)
    assert x.shape == (N_FULL, IN_DIM) and weight.shape == (NUM_B, IN_DIM * OUT_DIM)

    if _PROGRAM is None:
        _PROGRAM = _build_program()
    nc = _PROGRAM

    xtype32 = xtype.astype(np.int32)
    wcomp_host = np.ascontiguousarray(w_comp).astype(NPBF16)
    # packed weight [p, b, ih, o]
    wt_host = np.ascontiguousarray(
        weight.reshape(NUM_B, 2, P, OUT_DIM).transpose(2, 0, 1, 3).reshape(P, -1)
    ).astype(NPBF16)
    in_maps = []
    for c in range(N_CORES):
        s = slice(c * ROWS, (c + 1) * ROWS)
        in_maps.append(
            {
                "xT": np.ascontiguousarray(
                    x[s].T.reshape(2, P, ROWS).transpose(1, 0, 2).reshape(P, 2 * ROWS)
                ).astype(NPBF16),
                "xtype": np.ascontiguousarray(xtype32[s]),
                "wcomp": wcomp_host,
                "wt": wt_host,
            }
        )

    res = run_bass_kernel_spmd(nc, in_maps, list(range(N_CORES)), trace=trace)
    LAST_RESULT = res

    out = np.empty((N_FULL, OUT_DIM), np.float32)
    for c in range(N_CORES):
        s = slice(c * ROWS, (c + 1) * ROWS)
        out[s] = res.results[c]["outT"].T
    return out
